# revision 46
# baseline (speedup 1.0000x reference)
"""Trainium2 Bass kernel for batched single-head attention with seq-sum pooling.

Reference computation (B=16, S=2048, D=512, fp32):
    q = x @ W_q ; k = x @ W_k ; v = x @ W_v          per batch  [S, D]
    scores = q @ k.T / sqrt(D)                        [S, S]
    attn = softmax(scores, axis=-1)
    out_b = sum_s (attn @ v)[s, :]                    [D]

Key algebraic restructure: the final sum over query positions commutes with
the attn @ v matmul:
    out_b = (sum_q attn[q, :]) @ v = (r^T E) @ v
where E = exp(scores / sqrt(D)) and r[q] = 1 / rowsum_q(E).  This removes the
second [S,S]x[S,D] matmul (~36% of the FLOPs) and replaces it with one
[1,S]x[S,S] column-sum matmul plus one [1,S]x[S,D] matvec.

Sharding: pure data parallelism over batch — 2 batch elements per core on 8
NeuronCores, weights replicated, no collectives.  Host concatenates per-core
[2, D] outputs.

Matmul operands are bf16 (fp32 PSUM accumulation), which streams the PE at
~217 ns per [128x128]x[128x512] matmul and allows the X transpose to ride the
DMA crossbar (f32 -> bf16 cast-DMA to a DRAM scratch, then hardware
transpose-DMA into SBUF) instead of burning TensorE cycles — transpose-mode
matmuls also don't count as PE activity for the HAM clock gate and would
re-throttle the array to 1.2 GHz.  Measured end-to-end rel error ~2e-3
(reference tolerance 2e-2).  Emission is software-pipelined: batch 0's
w-phase weaves into batch 1's projections and batch 1's V projections weave
through the tail w-phase so the PE never idles long enough to go cold.
"""

import sys

sys.path.insert(0, "/opt/trn_rl_repo")

import numpy as np

import concourse.bass as bass
import concourse.mybir as mybir
import concourse.tile as tile
from concourse import bacc
from concourse.bass_utils import run_bass_kernel_spmd
from concourse.masks import make_identity

B, S, D = 16, 2048, 512
P = 128
N_CORES = 8
B_PER_CORE = B // N_CORES  # 2
SCALE = 1.0 / float(np.sqrt(D))

F32 = mybir.dt.float32
BF16 = mybir.dt.bfloat16

N_ST = S // P  # 16 s-tiles (partition tiles of the sequence dim)
N_DT = D // P  # 4 d-tiles (partition tiles of the feature dim)
NCH = 512  # moving free dim per matmul (one fp32 PSUM bank)
N_SC = S // NCH  # 4 s-chunks of the sequence dim
N_KC = S // NCH  # 4 k-chunks of the key dim


def build_nc():
    nc = bacc.Bacc("TRN2", target_bir_lowering=False, debug=False, num_devices=N_CORES)
    x_ext = nc.dram_tensor(
        "inputs", [B_PER_CORE, S, D], F32, kind="ExternalInput"
    ).ap()
    wq_ext = nc.dram_tensor("W_q", [D, D], F32, kind="ExternalInput").ap()
    wk_ext = nc.dram_tensor("W_k", [D, D], F32, kind="ExternalInput").ap()
    wv_ext = nc.dram_tensor("W_v", [D, D], F32, kind="ExternalInput").ap()
    out_ext = nc.dram_tensor("out", [B_PER_CORE, D], F32, kind="ExternalOutput").ap()

    with tile.TileContext(nc) as tc:
        with (
            tc.tile_pool(name="const", bufs=1) as const_pool,
            tc.tile_pool(name="w", bufs=1) as w_pool,
            tc.tile_pool(name="xnat", bufs=2) as xnat_pool,
            tc.tile_pool(name="xt", bufs=2) as xt_pool,
            tc.tile_pool(name="qkv", bufs=2) as qkv_pool,
            tc.tile_pool(name="e", bufs=4) as e_pool,
            tc.tile_pool(name="soft", bufs=4) as soft_pool,
            tc.tile_pool(name="wvec", bufs=2) as wvec_pool,
            tc.tile_pool(name="scps", bufs=2, space="PSUM") as sc_psum,
            tc.tile_pool(name="gpps", bufs=2, space="PSUM") as gp_psum,
            tc.tile_pool(name="wps", bufs=1, space="PSUM") as w_psum,
        ):
            one_t = const_pool.tile([1, 1], BF16)
            nc.gpsimd.memset(one_t[:], 1.0)
            ident_f = const_pool.tile([P, P], F32)
            make_identity(nc, ident_f[:])
            ident = const_pool.tile([P, P], BF16)
            nc.vector.tensor_copy(ident[:], ident_f[:])

            # x arrives via SWDGE cast-DMA (f32 -> bf16) into natural-layout
            # staging tiles; the transpose to xT happens on the PE as a
            # REGULAR identity matmul (out = x_tile.T @ I).  Unlike
            # transpose-mode matmuls, these pipeline their weight loads and
            # count as PE activity for the HAM clock gate, and unlike the DMA
            # crossbar transpose they don't serialize the DMA subsystem.
            def dma_x_chunk(b, sc, xnat_s):
                nc.gpsimd.dma_start(
                    out=xnat_s[:, sc * 4 : (sc + 1) * 4, :],
                    in_=x_ext[b, sc * NCH : (sc + 1) * NCH, :].rearrange(
                        "(t p) d -> p t d", p=P
                    ),
                )

            w_tiles = {}

            def dma_w(name, ext):
                w_s = w_pool.tile([P, N_DT, D], BF16, tag=name)
                nc.gpsimd.dma_start(
                    out=w_s[:], in_=ext.rearrange("(t p) e -> p t e", p=P)
                )
                w_tiles[name] = w_s

            # Batch 0's x chunks and the weight loads share the SWDGE queue;
            # order so each lands just before the PE needs it.
            xnat0_s = xnat_pool.tile([P, N_ST, D], BF16, tag="xnat")
            x0_loaded = [False] * N_SC
            # split the first chunk so the first transposes start sooner
            for t_i in range(4):
                nc.gpsimd.dma_start(
                    out=xnat0_s[:, t_i : t_i + 1, :],
                    in_=x_ext[0, t_i * P : (t_i + 1) * P, :].rearrange(
                        "(t p) d -> p t d", p=P
                    ),
                )
            x0_loaded[0] = True
            dma_w("wk", wk_ext)
            dma_x_chunk(0, 1, xnat0_s)
            x0_loaded[1] = True
            dma_w("wq", wq_ext)
            dma_x_chunk(0, 2, xnat0_s)
            x0_loaded[2] = True
            dma_w("wv", wv_ext)
            wk_s, wq_s, wv_s = w_tiles["wk"], w_tiles["wq"], w_tiles["wv"]

            # ---------- thunk builders (emission deferred for interleaving) --

            def proj_thunks(b, xnat_s, loaded):
                """Transpose + QT/KT projection thunks for batch b."""
                xt_s = xt_pool.tile([P, N_DT, S], BF16, tag="xt")
                qt_s = qkv_pool.tile([P, N_DT, S], BF16, tag="qt")
                kt_s = qkv_pool.tile([P, N_DT, S], BF16, tag="kt")

                def make_dma(sc):
                    def th():
                        dma_x_chunk(b, sc, xnat_s)

                    return th

                dma_th = [
                    None if loaded[sc] else make_dma(sc) for sc in range(N_SC)
                ]

                def make_trans_unit(sc, t_i):
                    def th():
                        st = sc * 4 + t_i
                        tp = sc_psum.tile([P, N_DT * P], F32, tag="sc")
                        for dt_i in range(N_DT):
                            nc.tensor.matmul(
                                tp[:, dt_i * P : (dt_i + 1) * P],
                                xnat_s[:, st, dt_i * P : (dt_i + 1) * P],
                                ident[:],
                                start=True,
                                stop=True,
                                skip_group_check=True,
                            )
                        nc.vector.tensor_copy(
                            xt_s[:, :, st * P : (st + 1) * P],
                            tp[:].rearrange("p (t c) -> p t c", t=N_DT),
                        )

                    return th

                trans_th = [
                    [make_trans_unit(sc, t_i) for t_i in range(4)]
                    for sc in range(N_SC)
                ]

                def make_kq(sc, w_src, dst, et):
                    def th():
                        mp = gp_psum.tile([P, NCH], F32, tag="gp")
                        for kd in range(N_DT):
                            nc.tensor.matmul(
                                mp[:],
                                w_src[:, kd, et * P : (et + 1) * P],
                                xt_s[:, kd, sc * NCH : (sc + 1) * NCH],
                                start=(kd == 0),
                                stop=(kd == N_DT - 1),
                            )
                        nc.vector.tensor_copy(
                            dst[:, et, sc * NCH : (sc + 1) * NCH], mp[:]
                        )

                    return th

                kq_th = [
                    [
                        make_kq(sc, w_src, dst, et)
                        for w_src, dst in ((wk_s, kt_s), (wq_s, qt_s))
                        for et in range(N_DT)
                    ]
                    for sc in range(N_SC)
                ]
                return (qt_s, kt_s), dma_th, trans_th, kq_th

            def emit_ltp(dma_th, trans_th, kq_th, extra=None):
                """Emit the transpose/projection stream: chunk sc+1's
                transposes weave between chunk sc's projection groups so the
                PE stream stays dense."""
                extra = list(extra) if extra else []
                ei = 0
                if dma_th[0] is not None:
                    dma_th[0]()
                    dma_th[0] = None
                for th in trans_th[0]:
                    th()
                for sc in range(N_SC):
                    for j in (sc + 1, sc + 2):
                        if j < N_SC and dma_th[j] is not None:
                            dma_th[j]()
                            dma_th[j] = None
                    nxt = trans_th[sc + 1] if sc + 1 < N_SC else []
                    groups = list(kq_th[sc])
                    ti = 0
                    for g_i, g in enumerate(groups):
                        g()
                        while ti < len(nxt) and ti * len(groups) < (g_i + 1) * len(nxt):
                            nxt[ti]()
                            ti += 1
                        if ei < len(extra):
                            extra[ei]()
                            ei += 1
                    while ti < len(nxt):
                        nxt[ti]()
                        ti += 1
                while ei < len(extra):
                    extra[ei]()
                    ei += 1

            def emit_scores_qt(qt_s, kt_s, qt):
                """scores + exp + rowsum + reciprocal for one q-tile."""
                e_t = e_pool.tile([P, S], BF16, tag="e")
                rsum = soft_pool.tile([P, N_KC], F32, tag="rsum")
                for kc in range(N_KC):
                    sp = sc_psum.tile([P, NCH], F32, tag="sc")
                    for et in range(N_DT):
                        nc.tensor.matmul(
                            sp[:],
                            qt_s[:, et, qt * P : (qt + 1) * P],
                            kt_s[:, et, kc * NCH : (kc + 1) * NCH],
                            start=(et == 0),
                            stop=(et == N_DT - 1),
                        )
                    nc.scalar.activation(
                        e_t[:, kc * NCH : (kc + 1) * NCH],
                        sp[:],
                        mybir.ActivationFunctionType.Exp,
                        scale=SCALE,
                        accum_out=rsum[:, kc : kc + 1],
                    )
                rtot = soft_pool.tile([P, 1], F32, tag="rtot")
                nc.vector.reduce_sum(rtot[:], rsum[:], axis=mybir.AxisListType.X)
                rrec = soft_pool.tile([P, 1], F32, tag="rrec")
                nc.vector.reciprocal(rrec[:], rtot[:])
                # M=1 matmuls issue ~25% slower than M=128 ones; broadcast r
                # across a full 128-wide stationary tile (every PSUM row then
                # equals r^T E) to keep the colsum at full rate.
                r_t = soft_pool.tile([P, P], BF16, tag="r")
                nc.vector.tensor_copy(r_t[:], rrec[:, 0:1].broadcast_to([P, P]))
                return e_t, r_t

            def emit_colsum_qt(w_ps, e_t, r_t, qt):
                """w_ps[:, kc, :] += bcast(r_qt)^T @ E_qt (every row = colsum)."""
                for kc in range(N_KC):
                    nc.tensor.matmul(
                        w_ps[:, kc, :],
                        r_t[:],
                        e_t[:, kc * NCH : (kc + 1) * NCH],
                        start=(qt == 0),
                        stop=(qt == N_ST - 1),
                        skip_group_check=True,
                    )

            def phase_scores(b, qt_s, kt_s, per_qt_extra=None):
                w_ps = w_psum.tile([P, N_KC, NCH], F32, tag="w")
                prev = None
                for qt in range(N_ST):
                    cur = emit_scores_qt(qt_s, kt_s, qt)
                    if prev is not None:
                        emit_colsum_qt(w_ps, prev[0], prev[1], qt - 1)
                    if per_qt_extra is not None and qt < len(per_qt_extra):
                        per_qt_extra[qt]()
                    prev = cur
                emit_colsum_qt(w_ps, prev[0], prev[1], N_ST - 1)
                return w_ps

            def final_thunks(b, w_ps, xnat_s):
                """w-phase thunks, using out = (w @ X) @ W_v so no V
                projection is ever materialized: 4 ACT copies of w, 16 (PE
                row->column transpose + DVE broadcast), 16 y-accumulation
                matmuls y = w @ X, then the tiny epilogue y @ W_v and the
                output copy + DMA.  Emitted interleaved by the caller."""
                w_sb = wvec_pool.tile([1, S], BF16, tag="wsb")
                y_ps = sc_psum.tile([P, NCH], F32, tag="sc")
                wt_pads = {}
                yt_pads = {}
                thunks = []

                def make_wcopy(kc):
                    def th():
                        nc.scalar.copy(
                            w_sb[:, kc * NCH : (kc + 1) * NCH], w_ps[0:1, kc, :]
                        )

                    return th

                def row_to_bcast_cols(src_row, pads, key, tag):
                    """[1,128] SBUF row chunk -> K=1 matmul -> [128,1] PSUM
                    column -> DVE broadcast to a [128,128] stationary tile."""
                    tp = gp_psum.tile([P, 1], F32, tag="gp")
                    nc.tensor.matmul(
                        tp[:], src_row, one_t[0:1, 0:1], start=True, stop=True
                    )
                    pad = wvec_pool.tile([P, P], BF16, tag=tag)
                    nc.vector.tensor_copy(pad[:], tp[:, 0:1].broadcast_to([P, P]))
                    pads[key] = pad

                def make_wtrans(kt):
                    def th():
                        row_to_bcast_cols(
                            w_sb[0:1, kt * P : (kt + 1) * P],
                            wt_pads, kt, f"wtp{kt % 4}",
                        )

                    return th

                def make_ymm(st):
                    def th():
                        nc.tensor.matmul(
                            y_ps[:],
                            wt_pads[st][:],
                            xnat_s[:, st, :],
                            start=(st == 0),
                            stop=(st == N_ST - 1),
                            skip_group_check=True,
                        )

                    return th

                def epilogue_th():
                    # y [1, D] -> o = y @ W_v  (4 K=1 transposes + 4 matmuls)
                    y_sb = wvec_pool.tile([1, NCH], BF16, tag="ysb")
                    nc.scalar.copy(y_sb[:], y_ps[0:1, :])
                    o_ps = gp_psum.tile([P, NCH], F32, tag="gp")
                    for c in range(N_DT):
                        row_to_bcast_cols(
                            y_sb[0:1, c * P : (c + 1) * P], yt_pads, c, f"ytp{c}"
                        )
                    for c in range(N_DT):
                        nc.tensor.matmul(
                            o_ps[:],
                            yt_pads[c][:],
                            wv_s[:, c, :],
                            start=(c == 0),
                            stop=(c == N_DT - 1),
                            skip_group_check=True,
                        )
                    o_sb = wvec_pool.tile([1, NCH], F32, tag="osb")
                    nc.scalar.copy(o_sb[:], o_ps[0:1, :])
                    nc.sync.dma_start(out=out_ext[b : b + 1, :], in_=o_sb[:])

                for kc in range(N_KC):
                    thunks.append(make_wcopy(kc))
                for kt in range(N_ST):
                    thunks.append(make_wtrans(kt))
                    if kt >= 3:
                        thunks.append(make_ymm(kt - 3))
                for st in range(N_ST - 3, N_ST):
                    thunks.append(make_ymm(st))
                thunks.append(epilogue_th)
                return thunks

            # ------------------------- emission ------------------------------

            # batch 0: transposes woven into projections
            h0, dma0, trans0, kq0 = proj_thunks(0, xnat0_s, x0_loaded)
            q0, k0 = h0
            emit_ltp(dma0, trans0, kq0)

            wps0 = phase_scores(0, q0, k0)

            # batch 1 transposes/projections with batch 0's w-phase woven in
            xnat1_s = xnat_pool.tile([P, N_ST, D], BF16, tag="xnat")
            h1, dma1, trans1, kq1 = proj_thunks(1, xnat1_s, [False] * N_SC)
            q1, k1 = h1
            emit_ltp(dma1, trans1, kq1, extra=final_thunks(0, wps0, xnat0_s))

            wps1 = phase_scores(1, q1, k1)

            for th in final_thunks(1, wps1, xnat1_s):
                th()

    nc.compile()
    return nc


_NC_CACHE = None


def _get_nc():
    global _NC_CACHE
    if _NC_CACHE is None:
        _NC_CACHE = build_nc()
    return _NC_CACHE


def make_in_maps(inputs, W_q, W_k, W_v):
    inputs = np.ascontiguousarray(np.asarray(inputs, dtype=np.float32))
    W_q = np.ascontiguousarray(np.asarray(W_q, dtype=np.float32))
    W_k = np.ascontiguousarray(np.asarray(W_k, dtype=np.float32))
    W_v = np.ascontiguousarray(np.asarray(W_v, dtype=np.float32))
    return [
        {
            "inputs": inputs[i * B_PER_CORE : (i + 1) * B_PER_CORE],
            "W_q": W_q,
            "W_k": W_k,
            "W_v": W_v,
        }
        for i in range(N_CORES)
    ]


def kernel(**inputs) -> np.ndarray:
    nc = _get_nc()
    in_maps = make_in_maps(
        inputs["inputs"], inputs["W_q"], inputs["W_k"], inputs["W_v"]
    )
    res = run_bass_kernel_spmd(nc, in_maps, core_ids=list(range(N_CORES)))
    return np.concatenate(
        [res.results[i]["out"] for i in range(N_CORES)], axis=0
    ).astype(np.float32)
